# revision 15
# baseline (speedup 1.0000x reference)
"""RBF kernel-expfamily scoring on 8 Trainium2 NeuronCores.

scores[b] = sum_j exp(-gamma * ||x_b - X_j||^2) * alpha_j

Restructured for precision + speed (float16 matmuls, 10-bit mantissa):

  scores_b = e^{-g*x2_b} * [ sum_j a'_j * (e^{2g*P_jb} - 1) + sum_j a'_j ]

  where P_jb = X_j . x_b,  a'_j = alpha_j * e^{-g*X2_j}.

The "- 1" centering keeps the matmul rhs values small (|t| ~ 0.05)
so the f16 rounding error is ~20x smaller than rounding e^{2gP}~1.

Sharding: db dimension (X rows / alpha) split across the 8 cores; each core
computes partial_b = sum_{j in shard} a'_j * t_jb; the host sums partials,
adds the exact constant sum_j a'_j, and scales by e^{-g*x2_b}.

Per-core device pipeline, tiled [128 db-rows x 512 batch-cols]:
  - PE matmul (f16): psum[j,b] = sum_f (2g*X^T)[f,j] * x^T[f,b]  (K=256, 2 chunks,
    two j-tiles batched into one 2-bank PSUM tile)
  - ACT: kt[j,b] = Exp(psum)     (one [128,1024] op per PSUM pair, f32 out)
  - DVE: tt[j,b] = kt - 1        (writes f16)
  - PE matmul (f16): score rows += a'[j128,1]^T @ tt[j128,b512], col-tiled
    at tile_position (0, 32*(j%4)) so 4 M=1 matmuls run concurrently in
    disjoint PE column groups; partial rows land on PSUM partitions 0/32/64/96
  - DVE copy -> SBUF, strided DMA of the 4 rows out; host sums rows+cores.
"""

import functools
from contextlib import ExitStack

import numpy as np

BATCH = 8192
DB = 16384
FEAT = 256
NCORES = 8
SHARD = DB // NCORES  # 2048
NB = 512  # batch tile (matmul free dim)
NBT = BATCH // NB  # 16 batch tiles
NJT = SHARD // 128  # 16 db tiles of 128 rows


@functools.lru_cache(maxsize=4)
def _build(reps=1):
    import concourse.bacc as bacc
    import concourse.mybir as mybir
    import concourse.tile as tile

    f32 = mybir.dt.float32
    f16 = mybir.dt.float16

    nc = bacc.Bacc("TRN2", target_bir_lowering=False, debug=False)

    xT = nc.declare_dram_parameter("xT", [FEAT, BATCH], f16, isOutput=False)
    XTs = nc.declare_dram_parameter("XTs", [FEAT, SHARD], f16, isOutput=False)
    alphj = nc.declare_dram_parameter("alphj", [128, NJT], f16, isOutput=False)
    out = nc.declare_dram_parameter("out", [4, BATCH], f32, isOutput=True)

    with ExitStack() as ctx:
        tc = ctx.enter_context(tile.TileContext(nc))
        singles = ctx.enter_context(tc.tile_pool(name="singles", bufs=1))
        xpool = ctx.enter_context(tc.tile_pool(name="xstream", bufs=3))
        kpool = ctx.enter_context(tc.tile_pool(name="ktiles", bufs=3))
        tpool = ctx.enter_context(tc.tile_pool(name="ttiles", bufs=3))
        pp = ctx.enter_context(tc.tile_pool(name="bigps", bufs=3, space="PSUM"))
        sp = ctx.enter_context(tc.tile_pool(name="scoreps", bufs=2, space="PSUM"))

        # Resident: X^T shard as [128, fchunk, j], scaled by 2*gamma on host.
        XT_sb = singles.tile([128, 2, SHARD], f16)
        nc.sync.dma_start(
            out=XT_sb, in_=XTs.rearrange("(c p) j -> p c j", p=128)
        )
        alph_sb = singles.tile([128, NJT], f16)
        nc.sync.dma_start(out=alph_sb, in_=alphj[:, :])
        spool = ctx.enter_context(tc.tile_pool(name="sout", bufs=2))

        for _rep in range(reps):
          for b in range(NBT):
            xt = xpool.tile([128, 2, NB], f16)
            nc.sync.dma_start(
                out=xt,
                in_=xT[:, b * NB : (b + 1) * NB].rearrange(
                    "(c p) n -> p c n", p=128
                ),
            )
            # 4 partial score rows at PSUM partitions 0/32/64/96 (col-tiled
            # M=1 matmuls in disjoint 32-col PE groups run concurrently).
            score_ps = sp.tile([128, NB], f32)
            for jp in range(NJT // 2):
                ps = pp.tile([128, 2, NB], f32)  # 2 PSUM banks
                for u in range(2):
                    j = jp * 2 + u
                    nc.tensor.matmul(
                        ps[:, u, :],
                        lhsT=XT_sb[:, 0, j * 128 : (j + 1) * 128],
                        rhs=xt[:, 0, :],
                        start=True,
                        stop=False,
                    )
                    nc.tensor.matmul(
                        ps[:, u, :],
                        lhsT=XT_sb[:, 1, j * 128 : (j + 1) * 128],
                        rhs=xt[:, 1, :],
                        start=False,
                        stop=True,
                    )
                kt = kpool.tile([128, 2, NB], f32)
                nc.scalar.activation(
                    kt, ps, mybir.ActivationFunctionType.Exp, bias=0.0, scale=1.0
                )
                tt = tpool.tile([128, 2, NB], f16)
                nc.vector.tensor_scalar_add(tt, kt, -1.0)
                for u in range(2):
                    j = jp * 2 + u
                    g = j % 4  # round-robin col group for PE concurrency
                    nc.tensor.matmul(
                        score_ps[32 * g : 32 * g + 1, :],
                        lhsT=alph_sb[:, j : j + 1],
                        rhs=tt[:, u, :],
                        start=(j < 4),
                        stop=(j >= NJT - 4),
                        tile_position=(0, 32 * g),
                    )
            s4 = spool.tile([128, NB], f32)
            nc.vector.tensor_copy(s4, score_ps)
            nc.sync.dma_start(
                out=out[:, b * NB : (b + 1) * NB], in_=s4[::32, :]
            )

    nc.compile()
    return nc


def _prep_inputs(x, X, alpha, gamma):
    x = np.ascontiguousarray(np.asarray(x, dtype=np.float32))
    X = np.ascontiguousarray(np.asarray(X, dtype=np.float32))
    alpha = np.asarray(alpha, dtype=np.float32).reshape(DB)
    g = float(np.asarray(gamma).reshape(-1)[0])

    x2 = np.einsum("bf,bf->b", x, x, dtype=np.float32)
    X2 = np.einsum("df,df->d", X, X, dtype=np.float32)

    xT = np.ascontiguousarray(x.T.astype(np.float16))  # [FEAT, BATCH]
    alphap = (alpha.astype(np.float64) * np.exp(-g * X2.astype(np.float64))).astype(
        np.float32
    )
    ex2 = np.exp(-g * x2.astype(np.float64))  # [BATCH], f64 host epilogue
    aconst = float(np.sum(alphap.astype(np.float64)))

    in_maps = []
    for i in range(NCORES):
        sl = slice(i * SHARD, (i + 1) * SHARD)
        XTs = np.ascontiguousarray(
            (np.float32(2.0 * g) * X[sl]).T.astype(np.float16)
        )
        alphj = np.ascontiguousarray(alphap[sl].reshape(NJT, 128).T.astype(np.float16))
        in_maps.append({"xT": xT, "XTs": XTs, "alphj": alphj})
    return in_maps, ex2, aconst


def run(x, X, alpha, gamma, trace=False, **spmd_kwargs):
    from concourse.bass_utils import run_bass_kernel_spmd

    nc = _build()
    in_maps, ex2, aconst = _prep_inputs(x, X, alpha, gamma)
    res = run_bass_kernel_spmd(
        nc, in_maps, list(range(NCORES)), trace=trace, **spmd_kwargs
    )
    total = np.zeros(BATCH, dtype=np.float64)
    for r in res.results:
        total += r["out"].reshape(4, BATCH).astype(np.float64).sum(axis=0)
    scores = (ex2 * (total + aconst)).astype(np.float32)
    return scores.reshape(BATCH, 1), res


def kernel(x, X, alpha, gamma):
    scores, _ = run(x, X, alpha, gamma, trace=False)
    return scores


# revision 16
# speedup vs baseline: 1.3945x; 1.3945x over previous
"""RBF kernel-expfamily scoring on 8 Trainium2 NeuronCores.

scores[b] = sum_j exp(-gamma * ||x_b - X_j||^2) * alpha_j

Restructured for precision + speed (float16 matmuls, 10-bit mantissa):

  scores_b = e^{-g*x2_b} * [ sum_j a'_j * (e^{2g*P_jb} - 1) + sum_j a'_j ]

  where P_jb = X_j . x_b,  a'_j = alpha_j * e^{-g*X2_j}.

The "- 1" centering keeps the matmul rhs values small (|t| ~ 0.05)
so the f16 rounding error is ~20x smaller than rounding e^{2gP}~1.

Sharding: db dimension (X rows / alpha) split across the 8 cores; each core
computes partial_b = sum_{j in shard} a'_j * t_jb; the host sums partials,
adds the exact constant sum_j a'_j, and scales by e^{-g*x2_b}.

Per-core device pipeline, tiled [128 db-rows x 512 batch-cols]:
  - PE matmul (f16): psum[j,b] = sum_f (2g*X^T)[f,j] * x^T[f,b]  (K=256, 2 chunks,
    two j-tiles batched into one 2-bank PSUM tile)
  - ACT: kt[j,b] = Exp(psum)     (one [128,1024] op per PSUM pair, f32 out)
  - DVE: tt[j,b] = kt - 1        (writes f16)
  - PE matmul (f16): score rows += a'[j128,1]^T @ tt[j128,b512], col-tiled
    at tile_position (0, 32*(j%4)) so 4 M=1 matmuls run concurrently in
    disjoint PE column groups; partial rows land on PSUM partitions 0/32/64/96
  - DVE copy -> SBUF, strided DMA of the 4 rows out; host sums rows+cores.
"""

import functools
from contextlib import ExitStack

import numpy as np

BATCH = 8192
DB = 16384
FEAT = 256
NCORES = 8
SHARD = DB // NCORES  # 2048
NB = 512  # batch tile (matmul free dim)
NBT = BATCH // NB  # 16 batch tiles
NJT = SHARD // 128  # 16 db tiles of 128 rows


@functools.lru_cache(maxsize=4)
def _build(reps=1):
    import concourse.bacc as bacc
    import concourse.mybir as mybir
    import concourse.tile as tile

    f32 = mybir.dt.float32
    f16 = mybir.dt.float16

    nc = bacc.Bacc("TRN2", target_bir_lowering=False, debug=False)

    xT = nc.declare_dram_parameter("xT", [FEAT, BATCH], f16, isOutput=False)
    XTs = nc.declare_dram_parameter("XTs", [FEAT, SHARD], f16, isOutput=False)
    alphj = nc.declare_dram_parameter("alphj", [128, NJT], f16, isOutput=False)
    out = nc.declare_dram_parameter("out", [4, BATCH], f32, isOutput=True)

    with ExitStack() as ctx:
        tc = ctx.enter_context(tile.TileContext(nc))
        singles = ctx.enter_context(tc.tile_pool(name="singles", bufs=1))
        xpool = ctx.enter_context(tc.tile_pool(name="xstream", bufs=3))
        kpool = ctx.enter_context(tc.tile_pool(name="ktiles", bufs=3))
        tpool = ctx.enter_context(tc.tile_pool(name="ttiles", bufs=3))
        pp = ctx.enter_context(tc.tile_pool(name="bigps", bufs=3, space="PSUM"))
        sp = ctx.enter_context(tc.tile_pool(name="scoreps", bufs=2, space="PSUM"))

        # Resident: X^T shard as [128, fchunk, j], scaled by 2*gamma on host.
        XT_sb = singles.tile([128, 2, SHARD], f16)
        nc.sync.dma_start(
            out=XT_sb, in_=XTs.rearrange("(c p) j -> p c j", p=128)
        )
        alph_sb = singles.tile([128, NJT], f16)
        nc.sync.dma_start(out=alph_sb, in_=alphj[:, :])
        spool = ctx.enter_context(tc.tile_pool(name="sout", bufs=2))

        # Warm-up exp on a dummy element: triggers the ~2.7us ACT table load
        # at t=0, concurrent with the initial DMAs, instead of stalling the
        # first real exp.
        warm = singles.tile([1, 1], f32)
        nc.vector.memset(warm, 0.0)
        nc.scalar.activation(
            warm, warm, mybir.ActivationFunctionType.Exp, bias=0.0, scale=1.0
        )

        for _rep in range(reps):
          for b in range(NBT):
            xt = xpool.tile([128, 2, NB], f16)
            nc.sync.dma_start(
                out=xt,
                in_=xT[:, b * NB : (b + 1) * NB].rearrange(
                    "(c p) n -> p c n", p=128
                ),
            )
            # 4 partial score rows at PSUM partitions 0/32/64/96 (col-tiled
            # M=1 matmuls in disjoint 32-col PE groups run concurrently).
            score_ps = sp.tile([128, NB], f32)
            for jp in range(NJT // 2):
                ps = pp.tile([128, 2, NB], f32)  # 2 PSUM banks
                for u in range(2):
                    j = jp * 2 + u
                    nc.tensor.matmul(
                        ps[:, u, :],
                        lhsT=XT_sb[:, 0, j * 128 : (j + 1) * 128],
                        rhs=xt[:, 0, :],
                        start=True,
                        stop=False,
                    )
                    nc.tensor.matmul(
                        ps[:, u, :],
                        lhsT=XT_sb[:, 1, j * 128 : (j + 1) * 128],
                        rhs=xt[:, 1, :],
                        start=False,
                        stop=True,
                    )
                kt = kpool.tile([128, 2, NB], f32)
                nc.scalar.activation(
                    kt, ps, mybir.ActivationFunctionType.Exp, bias=0.0, scale=1.0
                )
                tt = tpool.tile([128, 2, NB], f16)
                nc.vector.tensor_scalar_add(tt, kt, -1.0)
                for u in range(2):
                    j = jp * 2 + u
                    g = j % 4  # round-robin col group for PE concurrency
                    nc.tensor.matmul(
                        score_ps[32 * g : 32 * g + 1, :],
                        lhsT=alph_sb[:, j : j + 1],
                        rhs=tt[:, u, :],
                        start=(j < 4),
                        stop=(j >= NJT - 4),
                        tile_position=(0, 32 * g),
                    )
            s4 = spool.tile([128, NB], f32)
            nc.vector.tensor_copy(s4, score_ps)
            nc.sync.dma_start(
                out=out[:, b * NB : (b + 1) * NB], in_=s4[::32, :]
            )

    nc.compile()
    return nc


def _prep_inputs(x, X, alpha, gamma):
    x = np.ascontiguousarray(np.asarray(x, dtype=np.float32))
    X = np.ascontiguousarray(np.asarray(X, dtype=np.float32))
    alpha = np.asarray(alpha, dtype=np.float32).reshape(DB)
    g = float(np.asarray(gamma).reshape(-1)[0])

    x2 = np.einsum("bf,bf->b", x, x, dtype=np.float32)
    X2 = np.einsum("df,df->d", X, X, dtype=np.float32)

    xT = np.ascontiguousarray(x.T.astype(np.float16))  # [FEAT, BATCH]
    alphap = (alpha.astype(np.float64) * np.exp(-g * X2.astype(np.float64))).astype(
        np.float32
    )
    ex2 = np.exp(-g * x2.astype(np.float64))  # [BATCH], f64 host epilogue
    aconst = float(np.sum(alphap.astype(np.float64)))

    in_maps = []
    for i in range(NCORES):
        sl = slice(i * SHARD, (i + 1) * SHARD)
        XTs = np.ascontiguousarray(
            (np.float32(2.0 * g) * X[sl]).T.astype(np.float16)
        )
        alphj = np.ascontiguousarray(alphap[sl].reshape(NJT, 128).T.astype(np.float16))
        in_maps.append({"xT": xT, "XTs": XTs, "alphj": alphj})
    return in_maps, ex2, aconst


def run(x, X, alpha, gamma, trace=False, **spmd_kwargs):
    from concourse.bass_utils import run_bass_kernel_spmd

    nc = _build()
    in_maps, ex2, aconst = _prep_inputs(x, X, alpha, gamma)
    res = run_bass_kernel_spmd(
        nc, in_maps, list(range(NCORES)), trace=trace, **spmd_kwargs
    )
    total = np.zeros(BATCH, dtype=np.float64)
    for r in res.results:
        total += r["out"].reshape(4, BATCH).astype(np.float64).sum(axis=0)
    scores = (ex2 * (total + aconst)).astype(np.float32)
    return scores.reshape(BATCH, 1), res


def kernel(x, X, alpha, gamma):
    scores, _ = run(x, X, alpha, gamma, trace=False)
    return scores
